# revision 18
# baseline (speedup 1.0000x reference)
"""GNN message-passing kernel for Trainium2, SPMD across 8 NeuronCores.

Computation (per reference):
    m_e   = h[src_e] * (1 - d_e) + h[dst_e]
    agg   = segment_sum(m, dst)
    deg   = segment_sum(1, dst)
    h_new = where(deg > 0, agg, h)
    out   = relu(h_new @ W.T + b)

Algebraic form used on device (exact):
    aggT[f, v] = sum_{e: dst=v} (1-d_e) h[src_e, f]        (transposed)
    h_newT     = aggT + hscT,  hscT[f, v] = max(deg_v,1) * h[v, f]  (host)
    outT[o, v] = relu(sum_f W[o,f] h_newT[f,v] + b_o)

Distribution: dst nodes are assigned to 8*49=392 blocks of 128 lanes by a
degree-balanced serpentine (minimises the max edges per block); cores own 49
blocks each, no collectives.  Each core dma_gathers h[src] rows (bf16, 256B
descriptors, trailing -1 pads trimmed by the ucode) from a replicated bf16 h
with the 4 SWDGE queues rotating so descriptor generation overlaps across Q7
core pairs.  Per block, a selection matrix s01[e, v] = (1-d_e)*[lane(dst_e)=v]
is built in two batched broadcast tensor_tensor ops on DVE, and the PE
accumulates aggT via lhsT=gathered-tile matmuls into PSUM.  The 128x128
linear runs transposed with W^T as constant stationary weights; bias+relu is
a single scalar-engine activation.  deg and the deg*h term are computed on
the host (metadata-scale work), as is the final node permutation.

SPMD constraint: one NEFF for all 8 cores; per-(core,block) tile counts are
padded to the global max; all data-dependence lives in per-core input
tensors (idx, dstsh, om, hscT).
"""
import sys

if "/opt/trn_rl_repo" not in sys.path:
    sys.path.insert(0, "/opt/trn_rl_repo")

import ml_dtypes
import numpy as np

import concourse.bass as bass
import concourse.bacc as bacc
import concourse.mybir as mybir
import concourse.tile as tile
from concourse import bass_utils

N_CORES = 8
P = 128
GB_BUFS = 3
# Blocks gathered per dma_gather call.  Bounded by the SWDGE per-queue
# descriptor-ring carveout (~256 descs): descs_per_call = M*t*128/16 + 1
# must fit, or the ucode's await_space deadlocks.
M_BLK = 1
BF16 = ml_dtypes.bfloat16

_compiled = {}


def _build(n_nodes, npc_pad, nblk, t_e, t_o, t_tot):
    """Build + compile the SPMD Bass program.

    n_nodes: rows of the replicated bf16 gather table
    npc_pad: padded nodes per core (nblk * 128)
    nblk:    128-node blocks per core
    t_e/t_o: even/odd-parity gather tiles per block (uniform across cores)
    t_tot:   t_e + t_o
    """
    f32 = mybir.dt.float32
    bf16 = mybir.dt.bfloat16
    i16 = mybir.dt.int16

    nc = bacc.Bacc("TRN2", target_bir_lowering=False, debug=False,
                   num_devices=N_CORES, num_swdge_queues=4)

    hrep = nc.dram_tensor("hrep", [n_nodes, P], bf16, kind="ExternalInput")
    iota = nc.dram_tensor("iota", [P, P], bf16, kind="ExternalInput")
    wt = nc.dram_tensor("wt", [P, P], bf16, kind="ExternalInput")
    bvec = nc.dram_tensor("bvec", [P, 1], f32, kind="ExternalInput")
    hsct = nc.dram_tensor("hsct", [P, npc_pad], f32, kind="ExternalInput")
    idxe = nc.dram_tensor("idxe", [P, nblk * t_e * 8], i16, kind="ExternalInput")
    idxo = nc.dram_tensor("idxo", [P, nblk * t_o * 8], i16, kind="ExternalInput")
    dstsh = nc.dram_tensor("dstsh", [P, nblk * t_tot], bf16, kind="ExternalInput")
    omf = nc.dram_tensor("omf", [P, nblk * t_tot], f32, kind="ExternalInput")
    outt = nc.dram_tensor("outt", [P, npc_pad], bf16, kind="ExternalOutput")

    # Even/odd rows of the bf16 table as strided [n/2, 128] views (row
    # stride 256 elems = 512B): int16 gather indices address 50k rows as
    # idx = src >> 1.
    h_pairs = hrep[:].rearrange("(a b) f -> a b f", b=2)
    h_even = h_pairs[:, 0, :]
    h_odd = h_pairs[:, 1, :]

    with tile.TileContext(nc) as tc:
        with tc.tile_pool(name="const", bufs=1) as constp, \
             tc.tile_pool(name="meta", bufs=1) as metap, \
             tc.tile_pool(name="gbe", bufs=GB_BUFS) as gbep, \
             tc.tile_pool(name="gbo", bufs=GB_BUFS) as gbop, \
             tc.tile_pool(name="sel", bufs=4) as selp, \
             tc.tile_pool(name="gs", bufs=8) as gsp, \
             tc.tile_pool(name="blk", bufs=3) as blkp, \
             tc.tile_pool(name="psmm", bufs=2, space="PSUM") as psmm, \
             tc.tile_pool(name="psy", bufs=2, space="PSUM") as psy:

            # ---- one-time constants ----
            iota_sb = constp.tile([P, P], bf16)
            nc.sync.dma_start(out=iota_sb[:], in_=iota[:])
            wt_sb = constp.tile([P, P], bf16)
            nc.sync.dma_start(out=wt_sb[:], in_=wt[:])
            bias_sb = constp.tile([P, 1], f32)
            nc.sync.dma_start(out=bias_sb[:], in_=bvec[:])

            # ---- per-core metadata ----
            idxe_sb = metap.tile([P, nblk * t_e * 8], i16)
            nc.sync.dma_start(out=idxe_sb[:], in_=idxe[:])
            idxo_sb = metap.tile([P, nblk * t_o * 8], i16)
            nc.sync.dma_start(out=idxo_sb[:], in_=idxo[:])
            dstsh_sb = metap.tile([P, nblk * t_tot], bf16)
            nc.sync.dma_start(out=dstsh_sb[:], in_=dstsh[:])
            om_sb = metap.tile([P, nblk * t_tot], f32)
            nc.sync.dma_start(out=om_sb[:], in_=omf[:])
            hsct_sb = metap.tile([P, npc_pad], f32)
            nc.sync.dma_start(out=hsct_sb[:], in_=hsct[:])
            yout_sb = metap.tile([P, npc_pad], bf16)

            qn = 0
            for bs in range(0, nblk, M_BLK):
                nb = min(M_BLK, nblk - bs)
                # ---- gather nb blocks' h[src] rows (even / odd) ----
                ge = gbep.tile([P, nb * t_e * P], bf16, tag="ge")
                nc.gpsimd.dma_gather(
                    out_ap=ge[:].rearrange("p (g f) -> p g f", f=P),
                    in_ap=h_even,
                    idxs_ap=idxe_sb[:, bs * t_e * 8:(bs + nb) * t_e * 8],
                    num_idxs=nb * t_e * P,
                    num_idxs_reg=nb * t_e * P,
                    elem_size=P,
                    elem_step=2 * P,
                    queue_num=qn % 4,
                )
                qn += 1
                go = gbop.tile([P, nb * t_o * P], bf16, tag="go")
                nc.gpsimd.dma_gather(
                    out_ap=go[:].rearrange("p (g f) -> p g f", f=P),
                    in_ap=h_odd,
                    idxs_ap=idxo_sb[:, bs * t_o * 8:(bs + nb) * t_o * 8],
                    num_idxs=nb * t_o * P,
                    num_idxs_reg=nb * t_o * P,
                    elem_size=P,
                    elem_step=2 * P,
                    queue_num=qn % 4,
                )
                qn += 1

                for bl in range(nb):
                    blk = bs + bl

                    # ---- batched selection: s01[p,t,j] = (dstsh[p,t]==j) ----
                    s01 = selp.tile([P, t_tot * P], bf16)
                    s3 = s01[:].rearrange("p (t j) -> p t j", j=P)
                    cols = slice(blk * t_tot, (blk + 1) * t_tot)
                    iota_b = iota_sb[:].unsqueeze(1).broadcast_to([P, t_tot, P])
                    dst_b = dstsh_sb[:, cols].unsqueeze(2).broadcast_to(
                        [P, t_tot, P])
                    nc.vector.tensor_tensor(out=s3, in0=iota_b, in1=dst_b,
                                            op=mybir.AluOpType.is_equal)

                    # ---- aggT[f,v] += ((1-d) G_tile)[e,f]^T @ s01[e,v] ----
                    agg_ps = psmm.tile([P, P], f32, tag="ps_agg")
                    for t in range(t_tot):
                        if t < t_e:
                            gbf = ge[:, (bl * t_e + t) * P:
                                     (bl * t_e + t + 1) * P]
                        else:
                            to = t - t_e
                            gbf = go[:, (bl * t_o + to) * P:
                                     (bl * t_o + to + 1) * P]
                        # om-scale on the (otherwise idle) scalar engine
                        gsc = gsp.tile([P, P], bf16)
                        col = blk * t_tot + t
                        nc.scalar.activation(
                            gsc[:], gbf,
                            mybir.ActivationFunctionType.Copy,
                            scale=om_sb[:, col:col + 1])
                        nc.tensor.matmul(out=agg_ps[:], lhsT=gsc[:],
                                         rhs=s01[:, t * P:(t + 1) * P],
                                         start=(t == 0),
                                         stop=(t == t_tot - 1))

                    # ---- h_newT = aggT + hscT  (bf16, SBUF) ----
                    hnew = blkp.tile([P, P], bf16)
                    nc.vector.tensor_tensor(
                        out=hnew[:], in0=agg_ps[:],
                        in1=hsct_sb[:, blk * P:(blk + 1) * P],
                        op=mybir.AluOpType.add)

                    # ---- yT = W @ h_newT ; out = relu(yT + b) ----
                    y_ps = psy.tile([P, P], f32, tag="ps_y")
                    nc.tensor.matmul(out=y_ps[:], lhsT=wt_sb[:], rhs=hnew[:],
                                     start=True, stop=True)
                    nc.scalar.activation(yout_sb[:, blk * P:(blk + 1) * P],
                                         y_ps[:],
                                         mybir.ActivationFunctionType.Relu,
                                         bias=bias_sb[:])

            nc.sync.dma_start(out=outt[:], in_=yout_sb[:])

    # The tile scheduler reorders the gather instructions, and its DMASW
    # semaphore lanes rotate in FINAL stream order (i % 8) while each sem is
    # locked to one SWDGE queue.  Re-assign queue_num = lane % 4 in stream
    # order so every semaphore always sees the same queue.
    import concourse.bass_isa as bass_isa  # noqa: F401  (InstDMAGatherAnt)
    from concourse.tile_scheduler import DMAInst

    def _patch(insts, i=0):
        for inst in insts:
            if isinstance(inst, str) or not hasattr(inst, "engine"):
                continue
            d = getattr(inst, "descendants", None)
            if d:
                i = _patch(d, i)
            if inst.engine == mybir.EngineType.Pool and isinstance(inst, DMAInst):
                if isinstance(inst, mybir.InstDMAGatherAnt):
                    inst.queue_num = (i % 8) % 4
                i += 1
        return i

    for fnc in nc.m.functions:
        for blkb in fnc.blocks:
            _patch(list(blkb.instructions))

    nc.compile()
    return nc


def _wrap16(flat):
    """int16 index array -> [128, n/16] layout replicated across the 8
    Q7 core groups (index j lives at [j%16, j//16])."""
    cols = flat.size // 16
    return np.tile(flat.reshape(cols, 16).T, (8, 1)).copy()


def _balanced_blocks(deg, nblk_g):
    """Assign nodes to nblk_g blocks of exactly 128 lanes, balancing the
    total in-degree per block via a degree-sorted serpentine.

    Returns nodes_of [nblk_g, 128] (node id or -1 filler)."""
    n = deg.size
    order = np.argsort(-deg, kind="stable")
    cap = nblk_g * P
    padded = np.full(cap, -1, dtype=np.int64)
    padded[:n] = order
    mat = padded.reshape(P, nblk_g)           # [round, block]
    mat[1::2] = mat[1::2, ::-1]               # serpentine
    return np.ascontiguousarray(mat.T)        # [block, lane]


def kernel(h, d, src, dst, W, b):
    h = np.ascontiguousarray(h, dtype=np.float32)
    d = np.asarray(d, dtype=np.float32)
    src_i = np.asarray(src).astype(np.int64)
    dst_i = np.asarray(dst).astype(np.int64)
    Wf = np.ascontiguousarray(W, dtype=np.float32)
    bf = np.ascontiguousarray(b, dtype=np.float32)

    n_nodes = h.shape[0]
    assert n_nodes % (2 * N_CORES) == 0
    npc = n_nodes // N_CORES
    nblk = (npc + P - 1) // P
    npc_pad = nblk * P
    nblk_g = nblk * N_CORES

    # ---- host: degree, coefficient, balanced block assignment ----
    deg = np.bincount(dst_i, minlength=n_nodes).astype(np.float32)
    coef = np.maximum(deg, 1.0)
    nodes_of = _balanced_blocks(deg, nblk_g)          # [nblk_g, 128]
    node2block = np.empty(n_nodes, dtype=np.int64)
    node2lane = np.empty(n_nodes, dtype=np.int64)
    valid = nodes_of >= 0
    node2block[nodes_of[valid]] = np.repeat(
        np.arange(nblk_g), P).reshape(nblk_g, P)[valid]
    node2lane[nodes_of[valid]] = np.tile(np.arange(P), (nblk_g, 1))[valid]

    # ---- edges sorted by (block, parity, src) ----
    eb = node2block[dst_i]
    par = (src_i & 1).astype(np.int64)
    key = eb * 2 + par
    order_e = np.lexsort((src_i, key))
    eb_s = eb[order_e]
    par_s = par[order_e]
    key_s = key[order_e]
    src_s = src_i[order_e]
    lane_s = node2lane[dst_i[order_e]]
    om_s = 1.0 - d[order_e]

    # group bounds per (block, parity)
    bounds = np.searchsorted(key_s, np.arange(2 * nblk_g + 1))
    cnt = np.diff(bounds)                             # [2*nblk_g]
    cnt_e = cnt[0::2].reshape(N_CORES, nblk)
    cnt_o = cnt[1::2].reshape(N_CORES, nblk)
    t_e = max(1, int(np.max((cnt_e + P - 1) // P)))
    t_o = max(1, int(np.max((cnt_o + P - 1) // P)))
    t_tot = t_e + t_o

    key_comp = (n_nodes, npc_pad, nblk, t_e, t_o)
    if key_comp not in _compiled:
        _compiled[key_comp] = _build(n_nodes, npc_pad, nblk, t_e, t_o, t_tot)
    nc = _compiled[key_comp]

    # position of each edge within its (block, parity) group
    pos = np.arange(src_s.size) - bounds[key_s]

    # slot layout per core: idx arrays [nblk * t * 128], meta [128, nblk*t_tot]
    tiles_par = np.where(par_s == 0, 0, t_e)          # tile offset by parity
    col = eb_s % nblk * t_tot + tiles_par + pos // P  # global tile column
    row = pos % P
    core_e = eb_s // nblk

    h_bf = h.astype(BF16)
    iota_bf = np.tile(np.arange(P, dtype=np.float32)[None, :], (P, 1)).astype(BF16)
    wt_bf = np.ascontiguousarray(Wf.T).astype(BF16)
    bias_col = bf.reshape(P, 1).astype(np.float32)

    in_maps = []
    for c in range(N_CORES):
        mc = core_e == c
        par_c = par_s[mc]
        src_c = src_s[mc]
        lane_c = lane_s[mc]
        om_c = om_s[mc]
        colw = col[mc]
        roww = row[mc]
        posw = pos[mc]
        blkw = eb_s[mc] % nblk

        # pad slots gather row 0 (idx 0); their s01 columns select nothing
        # (dstsh -1, om 0), so the value is irrelevant.
        idxe_f = np.zeros(nblk * t_e * P, dtype=np.int16)
        idxo_f = np.zeros(nblk * t_o * P, dtype=np.int16)
        me = par_c == 0
        idxe_f[blkw[me] * (t_e * P) + posw[me]] = (src_c[me] >> 1).astype(np.int16)
        mo = ~me
        idxo_f[blkw[mo] * (t_o * P) + posw[mo]] = (src_c[mo] >> 1).astype(np.int16)

        dstsh_a = np.full((P, nblk * t_tot), -1.0, dtype=np.float32)
        om_a = np.zeros((P, nblk * t_tot), dtype=np.float32)
        dstsh_a[roww, colw] = lane_c.astype(np.float32)
        om_a[roww, colw] = om_c

        nodes_c = nodes_of[c * nblk:(c + 1) * nblk].reshape(-1)   # [npc_pad]
        vmask = nodes_c >= 0
        hsct_a = np.zeros((npc_pad, P), dtype=np.float32)
        hsct_a[vmask] = coef[nodes_c[vmask], None] * h[nodes_c[vmask]]

        in_maps.append({
            "hrep": h_bf, "iota": iota_bf, "wt": wt_bf, "bvec": bias_col,
            "hsct": np.ascontiguousarray(hsct_a.T),
            "idxe": _wrap16(idxe_f), "idxo": _wrap16(idxo_f),
            "dstsh": dstsh_a.astype(BF16), "omf": om_a,
        })

    res = bass_utils.run_bass_kernel_spmd(
        nc, in_maps, core_ids=list(range(N_CORES)))

    out = np.empty((n_nodes, P), dtype=np.float32)
    for c in range(N_CORES):
        nodes_c = nodes_of[c * nblk:(c + 1) * nblk].reshape(-1)
        vmask = nodes_c >= 0
        yc = res.results[c]["outt"].T.astype(np.float32)   # [npc_pad, 128]
        out[nodes_c[vmask]] = yc[vmask]
    return out


# revision 30
# speedup vs baseline: 1.2045x; 1.2045x over previous
"""GNN message-passing kernel for Trainium2, SPMD across 8 NeuronCores.

Computation (per reference):
    m_e   = h[src_e] * (1 - d_e) + h[dst_e]
    agg   = segment_sum(m, dst)
    deg   = segment_sum(1, dst)
    h_new = where(deg > 0, agg, h)
    out   = relu(h_new @ W.T + b)

Algebraic form used on device (exact):
    aggT[f, v] = sum_{e: dst=v} (1-d_e) h[src_e, f]        (transposed)
    h_newT     = aggT + hscT,  hscT[f, v] = max(deg_v,1) * h[v, f]  (host)
    outT[o, v] = relu(sum_f W[o,f] h_newT[f,v] + b_o)

Distribution: dst nodes are assigned to 8*49=392 blocks of 128 lanes by a
degree-balanced serpentine (minimises the max edges per block); cores own 49
blocks each, no collectives.  Each core dma_gathers h[src] rows (bf16, 256B
descriptors, trailing -1 pads trimmed by the ucode) from a replicated bf16 h
with the 4 SWDGE queues rotating so descriptor generation overlaps across Q7
core pairs.  Per block, a selection matrix s01[e, v] = (1-d_e)*[lane(dst_e)=v]
is built in two batched broadcast tensor_tensor ops on DVE, and the PE
accumulates aggT via lhsT=gathered-tile matmuls into PSUM.  The 128x128
linear runs transposed with W^T as constant stationary weights; bias+relu is
a single scalar-engine activation.  deg and the deg*h term are computed on
the host (metadata-scale work), as is the final node permutation.

SPMD constraint: one NEFF for all 8 cores; per-(core,block) tile counts are
padded to the global max; all data-dependence lives in per-core input
tensors (idx, dstsh, om, hscT).
"""
import sys

if "/opt/trn_rl_repo" not in sys.path:
    sys.path.insert(0, "/opt/trn_rl_repo")

import ml_dtypes
import numpy as np

import concourse.bass as bass
import concourse.bacc as bacc
import concourse.mybir as mybir
import concourse.tile as tile
from concourse import bass_utils

N_CORES = 8
P = 128
GB_BUFS = 8
# Blocks per s01-build group (DVE op batching).  Gather calls stay at one
# (block, parity) each -- the SWDGE ring carveout caps num_idxs at ~1008.
S_GRP = 4
BF16 = ml_dtypes.bfloat16

_compiled = {}


def _build(n_nodes, npc_pad, nblk, t_e, t_o, t_tot):
    """Build + compile the SPMD Bass program.

    n_nodes: rows of the replicated bf16 gather table
    npc_pad: padded nodes per core (nblk * 128)
    nblk:    128-node blocks per core
    t_e/t_o: even/odd-parity gather tiles per block (uniform across cores)
    t_tot:   t_e + t_o
    """
    f32 = mybir.dt.float32
    bf16 = mybir.dt.bfloat16
    i16 = mybir.dt.int16

    # The SWDGE descriptor-ring carveout is ~64 descs per queue (fixed by
    # the runtime, NOT by dynamic_dma_scratch_size or the queue count), so
    # a dma_gather call is limited to ~1008 idxs: one (block, parity) per
    # call.  Queue rotation must match the tile scheduler's DMASW proc
    # lanes -- patched post-scheduling below.
    nc = bacc.Bacc("TRN2", target_bir_lowering=False, debug=False,
                   num_devices=N_CORES, num_swdge_queues=4)

    hrep = nc.dram_tensor("hrep", [n_nodes, P], bf16, kind="ExternalInput")
    iota = nc.dram_tensor("iota", [P, P], bf16, kind="ExternalInput")
    wt = nc.dram_tensor("wt", [P, P], bf16, kind="ExternalInput")
    bvec = nc.dram_tensor("bvec", [P, 1], f32, kind="ExternalInput")
    hsct = nc.dram_tensor("hsct", [P, npc_pad], f32, kind="ExternalInput")
    idxe = nc.dram_tensor("idxe", [P, nblk * t_e * 8], i16, kind="ExternalInput")
    idxo = nc.dram_tensor("idxo", [P, nblk * t_o * 8], i16, kind="ExternalInput")
    dstsh = nc.dram_tensor("dstsh", [P, nblk * t_tot], bf16, kind="ExternalInput")
    omf = nc.dram_tensor("omf", [P, nblk * t_tot], bf16, kind="ExternalInput")
    outt = nc.dram_tensor("outt", [P, npc_pad], bf16, kind="ExternalOutput")

    # Even/odd rows of the bf16 table as strided [n/2, 128] views (row
    # stride 256 elems = 512B): int16 gather indices address 50k rows as
    # idx = src >> 1.
    h_pairs = hrep[:].rearrange("(a b) f -> a b f", b=2)
    h_even = h_pairs[:, 0, :]
    h_odd = h_pairs[:, 1, :]

    with tile.TileContext(nc) as tc:
        with tc.tile_pool(name="const", bufs=1) as constp, \
             tc.tile_pool(name="meta", bufs=1) as metap, \
             tc.tile_pool(name="gbe", bufs=GB_BUFS) as gbep, \
             tc.tile_pool(name="gbo", bufs=GB_BUFS) as gbop, \
             tc.tile_pool(name="sel", bufs=3) as selp, \
             tc.tile_pool(name="blk", bufs=3) as blkp, \
             tc.tile_pool(name="psmm", bufs=2, space="PSUM") as psmm, \
             tc.tile_pool(name="psy", bufs=2, space="PSUM") as psy:

            # ---- one-time constants ----
            iota_sb = constp.tile([P, P], bf16)
            nc.sync.dma_start(out=iota_sb[:], in_=iota[:])
            wt_sb = constp.tile([P, P], bf16)
            nc.sync.dma_start(out=wt_sb[:], in_=wt[:])
            bias_sb = constp.tile([P, 1], f32)
            nc.sync.dma_start(out=bias_sb[:], in_=bvec[:])

            # ---- per-core metadata ----
            idxe_sb = metap.tile([P, nblk * t_e * 8], i16)
            nc.sync.dma_start(out=idxe_sb[:], in_=idxe[:])
            idxo_sb = metap.tile([P, nblk * t_o * 8], i16)
            nc.sync.dma_start(out=idxo_sb[:], in_=idxo[:])
            dstsh_sb = metap.tile([P, nblk * t_tot], bf16)
            nc.sync.dma_start(out=dstsh_sb[:], in_=dstsh[:])
            om_sb = metap.tile([P, nblk * t_tot], bf16)
            nc.sync.dma_start(out=om_sb[:], in_=omf[:])
            hsct_sb = metap.tile([P, npc_pad], f32)
            nc.sync.dma_start(out=hsct_sb[:], in_=hsct[:])
            yout_sb = metap.tile([P, npc_pad], bf16)

            qn = 0
            for bs in range(0, nblk, S_GRP):
                nb = min(S_GRP, nblk - bs)
                # ---- batched selection build for the whole group:
                #   s01[p,t,j] = (dstsh[p,t] == j) * om[p,t]  (bf16) ----
                nt = nb * t_tot
                s01 = selp.tile([P, nt * P], bf16)
                s3 = s01[:].rearrange("p (t j) -> p t j", j=P)
                cols = slice(bs * t_tot, (bs + nb) * t_tot)
                iota_b = iota_sb[:].unsqueeze(1).broadcast_to([P, nt, P])
                dst_b = dstsh_sb[:, cols].unsqueeze(2).broadcast_to(
                    [P, nt, P])
                om_b = om_sb[:, cols].unsqueeze(2).broadcast_to(
                    [P, nt, P])
                nc.vector.tensor_tensor(out=s3, in0=iota_b, in1=dst_b,
                                        op=mybir.AluOpType.is_equal)
                nc.vector.tensor_tensor(out=s3, in0=s3, in1=om_b,
                                        op=mybir.AluOpType.mult)

                for bl in range(nb):
                    blk = bs + bl

                    # ---- gather this block's h[src] rows (even / odd) ----
                    ge = gbep.tile([P, t_e * P], bf16, tag="ge")
                    nc.gpsimd.dma_gather(
                        out_ap=ge[:].rearrange("p (g f) -> p g f", f=P),
                        in_ap=h_even,
                        idxs_ap=idxe_sb[:, blk * t_e * 8:(blk + 1) * t_e * 8],
                        num_idxs=t_e * P,
                        num_idxs_reg=t_e * P,
                        elem_size=P,
                        elem_step=2 * P,
                        queue_num=qn % 4,
                    )
                    qn += 1
                    go = gbop.tile([P, t_o * P], bf16, tag="go")
                    nc.gpsimd.dma_gather(
                        out_ap=go[:].rearrange("p (g f) -> p g f", f=P),
                        in_ap=h_odd,
                        idxs_ap=idxo_sb[:, blk * t_o * 8:(blk + 1) * t_o * 8],
                        num_idxs=t_o * P,
                        num_idxs_reg=t_o * P,
                        elem_size=P,
                        elem_step=2 * P,
                        queue_num=qn % 4,
                    )
                    qn += 1

                    # ---- aggT[f,v] += G_tile[e,f]^T @ s01_tile[e,v] ----
                    agg_ps = psmm.tile([P, P], f32, tag="ps_agg")
                    for t in range(t_tot):
                        if t < t_e:
                            gbf = ge[:, t * P:(t + 1) * P]
                        else:
                            to = t - t_e
                            gbf = go[:, to * P:(to + 1) * P]
                        st = (bl * t_tot + t) * P
                        nc.tensor.matmul(out=agg_ps[:], lhsT=gbf,
                                         rhs=s01[:, st:st + P],
                                         start=(t == 0),
                                         stop=(t == t_tot - 1))

                    # ---- h_newT = aggT + hscT  (bf16, SBUF) ----
                    hnew = blkp.tile([P, P], bf16)
                    nc.vector.tensor_tensor(
                        out=hnew[:], in0=agg_ps[:],
                        in1=hsct_sb[:, blk * P:(blk + 1) * P],
                        op=mybir.AluOpType.add)

                    # ---- yT = W @ h_newT ; out = relu(yT + b) ----
                    y_ps = psy.tile([P, P], f32, tag="ps_y")
                    nc.tensor.matmul(out=y_ps[:], lhsT=wt_sb[:], rhs=hnew[:],
                                     start=True, stop=True)
                    nc.scalar.activation(yout_sb[:, blk * P:(blk + 1) * P],
                                         y_ps[:],
                                         mybir.ActivationFunctionType.Relu,
                                         bias=bias_sb[:])

            nc.sync.dma_start(out=outt[:], in_=yout_sb[:])

    # The tile scheduler reorders the gather instructions, and its DMASW
    # semaphore lanes rotate in FINAL stream order (i % 8) while each sem is
    # locked to one SWDGE queue.  Re-assign queue_num = lane % 4 in stream
    # order so every semaphore always sees the same queue.
    import concourse.bass_isa as bass_isa  # noqa: F401  (InstDMAGatherAnt)
    from concourse.tile_scheduler import DMAInst

    def _patch(insts, i=0):
        for inst in insts:
            if isinstance(inst, str) or not hasattr(inst, "engine"):
                continue
            d = getattr(inst, "descendants", None)
            if d:
                i = _patch(d, i)
            if inst.engine == mybir.EngineType.Pool and isinstance(inst, DMAInst):
                if isinstance(inst, mybir.InstDMAGatherAnt):
                    inst.queue_num = (i % 8) % 4
                i += 1
        return i

    for fnc in nc.m.functions:
        for blkb in fnc.blocks:
            _patch(list(blkb.instructions))

    nc.compile()
    return nc


def _wrap16(flat):
    """int16 index array -> [128, n/16] layout replicated across the 8
    Q7 core groups (index j lives at [j%16, j//16])."""
    cols = flat.size // 16
    return np.tile(flat.reshape(cols, 16).T, (8, 1)).copy()


def _balanced_blocks(deg, nblk_g):
    """Assign nodes to nblk_g blocks of exactly 128 lanes, balancing the
    total in-degree per block via a degree-sorted serpentine.

    Returns nodes_of [nblk_g, 128] (node id or -1 filler)."""
    n = deg.size
    order = np.argsort(-deg, kind="stable")
    cap = nblk_g * P
    padded = np.full(cap, -1, dtype=np.int64)
    padded[:n] = order
    mat = padded.reshape(P, nblk_g)           # [round, block]
    mat[1::2] = mat[1::2, ::-1]               # serpentine
    return np.ascontiguousarray(mat.T)        # [block, lane]


def kernel(h, d, src, dst, W, b):
    h = np.ascontiguousarray(h, dtype=np.float32)
    d = np.asarray(d, dtype=np.float32)
    src_i = np.asarray(src).astype(np.int64)
    dst_i = np.asarray(dst).astype(np.int64)
    Wf = np.ascontiguousarray(W, dtype=np.float32)
    bf = np.ascontiguousarray(b, dtype=np.float32)

    n_nodes = h.shape[0]
    assert n_nodes % (2 * N_CORES) == 0
    npc = n_nodes // N_CORES
    nblk = (npc + P - 1) // P
    npc_pad = nblk * P
    nblk_g = nblk * N_CORES

    # ---- host: degree, coefficient, balanced block assignment ----
    deg = np.bincount(dst_i, minlength=n_nodes).astype(np.float32)
    coef = np.maximum(deg, 1.0)
    nodes_of = _balanced_blocks(deg, nblk_g)          # [nblk_g, 128]
    node2block = np.empty(n_nodes, dtype=np.int64)
    node2lane = np.empty(n_nodes, dtype=np.int64)
    valid = nodes_of >= 0
    node2block[nodes_of[valid]] = np.repeat(
        np.arange(nblk_g), P).reshape(nblk_g, P)[valid]
    node2lane[nodes_of[valid]] = np.tile(np.arange(P), (nblk_g, 1))[valid]

    # ---- edges sorted by (block, parity, src) ----
    eb = node2block[dst_i]
    par = (src_i & 1).astype(np.int64)
    key = eb * 2 + par
    order_e = np.lexsort((src_i, key))
    eb_s = eb[order_e]
    par_s = par[order_e]
    key_s = key[order_e]
    src_s = src_i[order_e]
    lane_s = node2lane[dst_i[order_e]]
    om_s = 1.0 - d[order_e]

    # group bounds per (block, parity)
    bounds = np.searchsorted(key_s, np.arange(2 * nblk_g + 1))
    cnt = np.diff(bounds)                             # [2*nblk_g]
    cnt_e = cnt[0::2].reshape(N_CORES, nblk)
    cnt_o = cnt[1::2].reshape(N_CORES, nblk)
    t_e = max(1, int(np.max((cnt_e + P - 1) // P)))
    t_o = max(1, int(np.max((cnt_o + P - 1) // P)))
    t_tot = t_e + t_o

    key_comp = (n_nodes, npc_pad, nblk, t_e, t_o)
    if key_comp not in _compiled:
        _compiled[key_comp] = _build(n_nodes, npc_pad, nblk, t_e, t_o, t_tot)
    nc = _compiled[key_comp]

    # position of each edge within its (block, parity) group
    pos = np.arange(src_s.size) - bounds[key_s]

    # slot layout per core: idx arrays [nblk * t * 128], meta [128, nblk*t_tot]
    tiles_par = np.where(par_s == 0, 0, t_e)          # tile offset by parity
    col = eb_s % nblk * t_tot + tiles_par + pos // P  # global tile column
    row = pos % P
    core_e = eb_s // nblk

    h_bf = h.astype(BF16)
    iota_bf = np.tile(np.arange(P, dtype=np.float32)[None, :], (P, 1)).astype(BF16)
    wt_bf = np.ascontiguousarray(Wf.T).astype(BF16)
    bias_col = bf.reshape(P, 1).astype(np.float32)

    in_maps = []
    for c in range(N_CORES):
        mc = core_e == c
        par_c = par_s[mc]
        src_c = src_s[mc]
        lane_c = lane_s[mc]
        om_c = om_s[mc]
        colw = col[mc]
        roww = row[mc]
        posw = pos[mc]
        blkw = eb_s[mc] % nblk

        # pad slots gather row 0 (idx 0); their s01 columns select nothing
        # (dstsh -1, om 0), so the value is irrelevant.
        idxe_f = np.zeros(nblk * t_e * P, dtype=np.int16)
        idxo_f = np.zeros(nblk * t_o * P, dtype=np.int16)
        me = par_c == 0
        idxe_f[blkw[me] * (t_e * P) + posw[me]] = (src_c[me] >> 1).astype(np.int16)
        mo = ~me
        idxo_f[blkw[mo] * (t_o * P) + posw[mo]] = (src_c[mo] >> 1).astype(np.int16)

        dstsh_a = np.full((P, nblk * t_tot), -1.0, dtype=np.float32)
        om_a = np.zeros((P, nblk * t_tot), dtype=np.float32)
        dstsh_a[roww, colw] = lane_c.astype(np.float32)
        om_a[roww, colw] = om_c

        nodes_c = nodes_of[c * nblk:(c + 1) * nblk].reshape(-1)   # [npc_pad]
        vmask = nodes_c >= 0
        hsct_a = np.zeros((npc_pad, P), dtype=np.float32)
        hsct_a[vmask] = coef[nodes_c[vmask], None] * h[nodes_c[vmask]]

        in_maps.append({
            "hrep": h_bf, "iota": iota_bf, "wt": wt_bf, "bvec": bias_col,
            "hsct": np.ascontiguousarray(hsct_a.T),
            "idxe": _wrap16(idxe_f), "idxo": _wrap16(idxo_f),
            "dstsh": dstsh_a.astype(BF16), "omf": om_a.astype(BF16),
        })

    res = bass_utils.run_bass_kernel_spmd(
        nc, in_maps, core_ids=list(range(N_CORES)))

    out = np.empty((n_nodes, P), dtype=np.float32)
    for c in range(N_CORES):
        nodes_c = nodes_of[c * nblk:(c + 1) * nblk].reshape(-1)
        vmask = nodes_c >= 0
        yc = res.results[c]["outt"].T.astype(np.float32)   # [npc_pad, 128]
        out[nodes_c[vmask]] = yc[vmask]
    return out
